# revision 1
# baseline (speedup 1.0000x reference)
"""Trainium2 Bass kernel for 3-layer GCN + Linear + log_softmax.

nn_GCN3_Lin1: x[100000,16], edge_index[2,6400000] ->
  h = relu(gcn(x;W1,b1)); h = relu(gcn(h;W2,b2)); h = relu(gcn(h;W3,b3))
  out = log_softmax(h @ Wf + bf)

Strategy (8 NeuronCores, graph/data parallel over nodes):
  Per-edge message passing uses the Q7 extended instructions ap_gather
  (SBUF->SBUF indexed read) + scatter_add (SBUF indexed accumulate),
  which run on all 8 Q7 cores in parallel and move data directly
  through the tensor ports -- no per-edge DMA descriptors.

  - 100352 padded nodes; core c owns blocks [12544j+1568c, +1568) of
    every window j; own-local slot i = 1568j + m.  Band j of the SBUF
    node table (partitions [16j,16j+16)) holds core j's nodes as a bf16
    table [16, 12545, 2] (row 12544 is a zero pad row): partition 16j+r
    holds features (r, r+16).
  - Per layer: feature-lane GEMM (lhsT=W halves) -> u = dinv*h, bf16
    pair-packed shard -> AllGather -> SBUF table.  Edges (dst owned by
    this core) are banded by owner(src); each band's edges are grouped
    into runs of 4 per dst (zero-row padded), gathered in one batched
    lib6 phase (ap_gather), 4:1 pre-reduced on DVE, spilled to DRAM,
    then accumulated in one batched lib3 phase (scatter_add) into
    acc[16j+r, dst_slot] -- batching avoids Q7 library thrash.
  - Group streams are pass-major (round-robin over dst) with >=32
    groups between same-dst repeats to dodge the scatter_add RMW
    pipeline hazard (empirically unsafe below ~24).
  - Band partials are summed with one tiny matmul (ones selector),
    z = dinv*S + b, relu -> next layer's feature-lane activations.
  - Self-loop is an explicit edge; dinv is host-precomputed.
"""

import math

import numpy as np
from ml_dtypes import bfloat16

from concourse import bass, mybir, bacc, tile
from concourse.bass_utils import run_bass_kernel_spmd

F32 = mybir.dt.float32
BF16 = mybir.dt.bfloat16
I16 = mybir.dt.int16

N_CORES = 8
WIN = 12544          # own nodes per core (= one table band)
BLK = 1568           # block of own nodes per (core, window)
NPAD = WIN * 8       # 100352
ZROW = WIN           # zero pad row in the table
GCH = 768            # groups per call (3072 gather idxs)
ACC_ROWS = 12800     # accumulator rows (12544 real + junk/pad)
JUNK = 12799
PACK_W = 448         # nodes per pack chunk (28 chunks)
NPACK = WIN // PACK_W
FIN_W = 256          # nodes per z-finish chunk (49 chunks)
NFIN = WIN // FIN_W
MIN_PASS = 32        # min group distance between same-dst repeats

LAST_RUN_INFO = {}


class _Plan:
    pass


def _owner(g):
    return (g % WIN) // BLK


def _ownloc(g):
    return BLK * (g // WIN) + (g % BLK)


def _build_plan(edge_index, n_nodes, n_cores=N_CORES):
    src = np.asarray(edge_index[0], dtype=np.int64)
    dst = np.asarray(edge_index[1], dtype=np.int64)
    assert n_nodes <= NPAD

    deg = np.bincount(dst, minlength=NPAD).astype(np.float64) + 1.0
    dinv = (1.0 / np.sqrt(deg)).astype(np.float32)

    own = np.arange(NPAD, dtype=np.int64)
    all_src = np.concatenate([src, own])
    all_dst = np.concatenate([dst, own])

    oc = _owner(all_dst)
    band = _owner(all_src)
    ap_i = _ownloc(all_src)
    sc_i = _ownloc(all_dst)

    ap_streams = [[None] * 8 for _ in range(n_cores)]
    sc_streams = [[None] * 8 for _ in range(n_cores)]
    max_g = 0
    for c in range(n_cores):
        mc = oc == c
        b_c, a_c, s_c = band[mc], ap_i[mc], sc_i[mc]
        for j in range(8):
            mj = b_c == j
            a, s = a_c[mj], s_c[mj]
            o = np.argsort(s, kind="stable")
            ss, aa = s[o], a[o]
            starts = np.searchsorted(ss, np.arange(WIN))
            runpos = np.arange(len(ss)) - starts[ss]
            cnt = np.bincount(ss, minlength=WIN)
            gcnt = -(-cnt // 4)
            off = np.zeros(WIN + 1, np.int64)
            off[1:] = np.cumsum(4 * gcnt)
            apad = np.full(off[-1], ZROW, np.int64)
            apad[off[ss] + runpos] = aa
            G = int(gcnt.sum())
            gdst = np.repeat(np.arange(WIN), gcnt)
            gocc = np.concatenate(
                [np.arange(n) for n in gcnt if n > 0]
            ) if G else np.zeros(0, np.int64)
            # pass-major order, sorted by dst within pass
            o2 = np.lexsort((gdst, gocc))
            gdst_p, gocc_p = gdst[o2], gocc[o2]
            gstart = (off[gdst_p] + 4 * gocc_p)  # start of group's 4 edges
            counts = np.bincount(gocc_p) if G else np.array([], np.int64)
            ap_parts, sc_parts = [], []
            pos = 0
            for r, n_r in enumerate(counts):
                if r > 0 and n_r < MIN_PASS:
                    pad = MIN_PASS - int(n_r)
                    ap_parts.append(np.full(pad * 4, ZROW, np.int64))
                    sc_parts.append(np.full(pad, JUNK, np.int64))
                gs = gstart[pos : pos + n_r]
                ap_parts.append(
                    apad[gs[:, None] + np.arange(4)[None, :]].reshape(-1)
                )
                sc_parts.append(gdst_p[pos : pos + n_r])
                pos += int(n_r)
            if ap_parts:
                a_full = np.concatenate(ap_parts)
                s_full = np.concatenate(sc_parts)
            else:
                a_full = np.zeros(0, np.int64)
                s_full = np.zeros(0, np.int64)
            ap_streams[c][j] = a_full
            sc_streams[c][j] = s_full
            max_g = max(max_g, len(s_full))

    ncall = max(1, math.ceil(max_g / GCH))
    # tail-trimmed group count of the last call (multiple of 16)
    last_g = max_g - (ncall - 1) * GCH
    last_g = min(GCH, ((last_g + 15) // 16) * 16)
    Gtot = (ncall - 1) * GCH + last_g

    apidx = np.full((n_cores, 8, Gtot * 4), ZROW, np.int16)
    scidx = np.full((n_cores, 8, Gtot), JUNK, np.int16)
    for c in range(n_cores):
        for j in range(8):
            a, s = ap_streams[c][j], sc_streams[c][j]
            apidx[c, j, : len(a)] = a
            scidx[c, j, : len(s)] = s

    def wrap(arr):
        L = arr.shape[2]
        return np.ascontiguousarray(
            arr.reshape(n_cores, 8, L // 16, 16).transpose(0, 1, 3, 2).reshape(
                n_cores, 128, L // 16
            )
        )

    pl = _Plan()
    pl.n_cores = n_cores
    pl.n_nodes = n_nodes
    pl.ncall = ncall
    pl.last_g = last_g
    pl.Gtot = Gtot
    pl.apidx = wrap(apidx)
    pl.scidx = wrap(scidx)
    pl.dinv = dinv
    return pl


def _make_in_maps(pl, x, W1, b1, W2, b2, W3, b3, Wf, bf):
    n = x.shape[0]
    xpad = np.zeros((NPAD, 16), np.float32)
    xpad[:n] = np.asarray(x, np.float32)

    jj = np.arange(WIN) // BLK
    mm = np.arange(WIN) % BLK

    def wpad(W):
        a, b_ = W.shape
        out = np.zeros((32, 32), np.float32)
        out[:a, :b_] = W
        return out.astype(bfloat16)

    Wfp = np.zeros((32, 6), np.float32)
    Wfp[:12] = np.asarray(Wf, np.float32)
    Wfp = Wfp.astype(bfloat16)

    def bpad(b_):
        out = np.zeros(32, np.float32)
        out[: len(b_)] = b_
        return out

    b3col = np.stack([bpad(b1), bpad(b2), bpad(b3)], axis=1).astype(np.float32)
    b3lo = np.ascontiguousarray(b3col[:16])
    b3hi = np.ascontiguousarray(b3col[16:])
    bfb = np.tile(np.asarray(bf, np.float32)[None, :], (128, 1))

    Bsel = np.zeros((128, 16), np.float32)
    for j in range(8):
        for r in range(16):
            Bsel[16 * j + r, r] = 1.0
    Bsel = Bsel.astype(bfloat16)

    in_maps = []
    for c in range(pl.n_cores):
        g = WIN * jj + BLK * c + mm
        xT = np.zeros((32, WIN), np.float32)
        xT[:16] = xpad[g].T
        d16 = np.tile(pl.dinv[g][None, :], (16, 1)).astype(bfloat16)
        in_maps.append(
            {
                "xT_in": xT.astype(bfloat16),
                "dinv_in": np.ascontiguousarray(d16),
                "apidx_in": pl.apidx[c],
                "scidx_in": pl.scidx[c],
                "W1": wpad(W1),
                "W2": wpad(W2),
                "W3": wpad(W3),
                "Wf": np.ascontiguousarray(Wfp),
                "b3lo": b3lo,
                "b3hi": b3hi,
                "bfb": bfb,
                "Bsel": np.ascontiguousarray(Bsel),
            }
        )
    return in_maps


def _assemble_output(pl, outs_per_core, d_out):
    full = np.empty((pl.n_nodes, d_out), dtype=np.float32)
    jj = np.arange(WIN) // BLK
    mm = np.arange(WIN) % BLK
    for c in range(pl.n_cores):
        gg = WIN * jj + BLK * c + mm
        keep = gg < pl.n_nodes
        full[gg[keep]] = outs_per_core[c][keep]
    return full


# ---------------------------------------------------------------------------
# Device kernel
# ---------------------------------------------------------------------------
def _build_kernel(pl):
    NC = pl.n_cores
    NCALL = pl.ncall
    LAST_G = pl.last_g
    APCOL = pl.Gtot * 4 // 16  # apidx cols per partition
    SCCOL = pl.Gtot // 16

    nc = bacc.Bacc("TRN2", target_bir_lowering=False, debug=False, num_devices=NC)

    xT_in = nc.dram_tensor("xT_in", [32, WIN], BF16, kind="ExternalInput")
    dinv_in = nc.dram_tensor("dinv_in", [16, WIN], BF16, kind="ExternalInput")
    apidx_in = nc.dram_tensor("apidx_in", [128, APCOL], I16, kind="ExternalInput")
    scidx_in = nc.dram_tensor("scidx_in", [128, SCCOL], I16, kind="ExternalInput")
    Ws = {
        n: nc.dram_tensor(n, [32, 32], BF16, kind="ExternalInput")
        for n in ("W1", "W2", "W3")
    }
    Wf_in = nc.dram_tensor("Wf", [32, 6], BF16, kind="ExternalInput")
    b3lo_in = nc.dram_tensor("b3lo", [16, 3], F32, kind="ExternalInput")
    b3hi_in = nc.dram_tensor("b3hi", [16, 3], F32, kind="ExternalInput")
    bfb_in = nc.dram_tensor("bfb", [128, 6], F32, kind="ExternalInput")
    Bsel_in = nc.dram_tensor("Bsel", [128, 16], BF16, kind="ExternalInput")
    out_dram = nc.dram_tensor("out", [WIN, 6], F32, kind="ExternalOutput")

    shard = nc.dram_tensor("shard", [16, WIN * 2], BF16)
    table = nc.dram_tensor("table", [NC * 16, WIN * 2], BF16, addr_space="Shared")
    msgd = nc.dram_tensor("msgd", [128, pl.Gtot * 4 * 2], BF16)
    rgroups = [list(range(NC))]

    with tile.TileContext(nc, num_cores=NC) as tc:
        with (
            tc.tile_pool(name="persist", bufs=1) as pers,
            tc.tile_pool(name="msg", bufs=2) as mpool,
            tc.tile_pool(name="red", bufs=3) as rpool,
            tc.tile_pool(name="idx", bufs=4) as ipool,
            tc.tile_pool(name="work", bufs=2) as wpool,
            tc.tile_pool(name="fin", bufs=2) as fpool,
            tc.tile_pool(name="ps", bufs=4, space="PSUM") as ppool,
            tc.tile_pool(name="ps2", bufs=4, space="PSUM") as ppool2,
        ):
            xT = pers.tile([32, WIN], BF16)
            nc.sync.dma_start(xT[:], xT_in[:, :])
            dinv16 = pers.tile([16, WIN], BF16)
            nc.sync.dma_start(dinv16[:], dinv_in[:, :])
            W_sb = {}
            for n in ("W1", "W2", "W3"):
                W_sb[n] = pers.tile([32, 32], BF16, name=f"Wsb_{n}")
                nc.sync.dma_start(W_sb[n][:], Ws[n][:, :])
            Wf_sb = pers.tile([32, 6], BF16)
            nc.sync.dma_start(Wf_sb[:], Wf_in[:, :])
            b3lo_sb = pers.tile([16, 3], F32)
            nc.sync.dma_start(b3lo_sb[:], b3lo_in[:, :])
            b3hi_sb = pers.tile([16, 3], F32)
            nc.sync.dma_start(b3hi_sb[:], b3hi_in[:, :])
            bfb_sb = pers.tile([128, 6], F32)
            nc.sync.dma_start(bfb_sb[:], bfb_in[:, :])
            Bsel_sb = pers.tile([128, 16], BF16)
            nc.sync.dma_start(Bsel_sb[:], Bsel_in[:, :])

            tab = pers.tile([128, (WIN + 1) * 2], BF16)
            nc.vector.memset(tab[:, WIN * 2 : (WIN + 1) * 2], 0.0)
            acc = pers.tile([128, ACC_ROWS * 2], BF16)
            out_sb = pers.tile([128, (WIN // 128) * 6], F32)

            for k, wname in enumerate(("W1", "W2", "W3")):
                # ---- pack: u = dinv * (x @ W), bf16 pair-packed shard ----
                for t in range(NPACK):
                    sl = slice(PACK_W * t, PACK_W * (t + 1))
                    ps_lo = ppool.tile([16, PACK_W], F32, space="PSUM",
                                       name=f"pl_{k}_{t}", tag="ps")
                    nc.tensor.matmul(
                        ps_lo[:], lhsT=W_sb[wname][:, 0:16], rhs=xT[:, sl],
                        start=True, stop=True,
                    )
                    ps_hi = ppool.tile([16, PACK_W], F32, space="PSUM",
                                       name=f"ph_{k}_{t}", tag="ps")
                    nc.tensor.matmul(
                        ps_hi[:], lhsT=W_sb[wname][:, 16:32], rhs=xT[:, sl],
                        start=True, stop=True,
                    )
                    dvf = wpool.tile([16, PACK_W], F32,
                                     name=f"dv_{k}_{t}", tag="dv")
                    nc.vector.tensor_copy(dvf[:], dinv16[:, sl])
                    pk = wpool.tile([16, PACK_W * 2], BF16,
                                    name=f"pk_{k}_{t}", tag="pk")
                    pkv = pk[:].rearrange("p (m w) -> p m w", w=2)
                    nc.vector.tensor_mul(pkv[:, :, 0:1], ps_lo[:], dvf[:])
                    nc.vector.tensor_mul(pkv[:, :, 1:2], ps_hi[:], dvf[:])
                    nc.sync.dma_start(
                        shard[:, PACK_W * 2 * t : PACK_W * 2 * (t + 1)], pk[:]
                    )
                nc.gpsimd.collective_compute(
                    "AllGather",
                    mybir.AluOpType.bypass,
                    replica_groups=rgroups,
                    ins=[shard.ap().opt()],
                    outs=[table.ap().opt()],
                )
                nc.sync.dma_start(tab[:, : WIN * 2], table[:, :])
                nc.vector.memset(acc[:], 0.0)

                # ---- phase 1 (lib6): batched gathers + 4:1 DVE reduce ----
                for t in range(NCALL):
                    g_t = GCH if t < NCALL - 1 else LAST_G
                    ia = ipool.tile([128, GCH * 4 // 16], I16,
                                    name=f"ia_{k}_{t}", tag="ia")
                    nc.sync.dma_start(
                        ia[:, : g_t * 4 // 16],
                        apidx_in[:, (GCH * 4 // 16) * t :
                                 (GCH * 4 // 16) * t + g_t * 4 // 16],
                    )
                    mg = mpool.tile([128, GCH * 4 * 2], BF16,
                                    name=f"mg_{k}_{t}", tag="mg")
                    nc.gpsimd.ap_gather(
                        out_ap=mg[:].rearrange("p (i w) -> p i w", w=2)[
                            :, : g_t * 4, :
                        ],
                        in_ap=tab[:].rearrange("p (e w) -> p e w", w=2),
                        idxs_ap=ia[:, : g_t * 4 // 16],
                        channels=128, num_elems=WIN + 1, d=2, num_idxs=g_t * 4,
                    )
                    nc.sync.dma_start(
                        msgd[:, GCH * 8 * t : GCH * 8 * t + g_t * 8],
                        mg[:, : g_t * 8],
                    )

                # ---- phase 2 (lib3): batched scatter-adds ----
                for t in range(NCALL):
                    g_t = GCH if t < NCALL - 1 else LAST_G
                    isx = ipool.tile([128, GCH // 16], I16,
                                     name=f"is_{k}_{t}", tag="is")
                    nc.sync.dma_start(
                        isx[:, : g_t // 16],
                        scidx_in[:, (GCH // 16) * t : (GCH // 16) * t + g_t // 16],
                    )
                    mgl = mpool.tile([128, GCH * 4 * 2], BF16,
                                     name=f"ml_{k}_{t}", tag="mg")
                    nc.sync.dma_start(
                        mgl[:, : g_t * 8],
                        msgd[:, GCH * 8 * t : GCH * 8 * t + g_t * 8],
                    )
                    rgs = rpool.tile([128, GCH * 2], BF16,
                                     name=f"rs_{k}_{t}", tag="rg")
                    with nc.allow_low_precision(
                        reason="4-element bf16 group pre-sums"
                    ):
                        nc.vector.tensor_reduce(
                            out=rgs[:].rearrange("p (g w) -> p g w", w=2)[
                                :, :g_t, :
                            ],
                            in_=mgl[:].rearrange("p (g e w) -> p g w e", e=4, w=2)[
                                :, :g_t, :, :
                            ],
                            axis=mybir.AxisListType.X,
                            op=mybir.AluOpType.add,
                        )
                    nc.gpsimd.scatter_add(
                        in_ap=acc[:].rearrange("p (e w) -> p e w", w=2),
                        idxs_ap=isx[:, : g_t // 16],
                        add_ap=rgs[:].rearrange("p (i w) -> p i w", w=2)[
                            :, :g_t, :
                        ],
                        channels=128, num_elems=ACC_ROWS, d=2, num_idxs=g_t,
                    )

                # ---- band-sum + z = dinv*S + b, relu -> next xT ----
                for t in range(NFIN):
                    ps2 = ppool2.tile([16, FIN_W * 2], F32, space="PSUM",
                                      name=f"ps2_{k}_{t}", tag="ps2")
                    nc.tensor.matmul(
                        ps2[:], lhsT=Bsel_sb[:],
                        rhs=acc[:, FIN_W * 2 * t : FIN_W * 2 * (t + 1)],
                        start=True, stop=True,
                    )
                    nsl = slice(FIN_W * t, FIN_W * (t + 1))
                    dvf2 = fpool.tile([16, FIN_W], F32,
                                      name=f"dz_{k}_{t}", tag="dz")
                    nc.vector.tensor_copy(dvf2[:], dinv16[:, nsl])
                    tmp = fpool.tile([16, FIN_W * 2], F32,
                                     name=f"tmp_{k}_{t}", tag="tmp")
                    p2v = ps2[:].rearrange("p (m w) -> p m w", w=2)
                    nc.vector.tensor_mul(tmp[:, 0:FIN_W],
                                         p2v[:, :, 0:1], dvf2[:])
                    nc.vector.tensor_mul(tmp[:, FIN_W : 2 * FIN_W],
                                         p2v[:, :, 1:2], dvf2[:])
                    zz = fpool.tile([16, FIN_W * 2], BF16,
                                    name=f"zz_{k}_{t}", tag="zz")
                    nc.scalar.activation(
                        out=zz[:, 0:FIN_W], in_=tmp[:, 0:FIN_W],
                        func=mybir.ActivationFunctionType.Relu,
                        bias=b3lo_sb[:, k : k + 1], scale=1.0,
                    )
                    nc.scalar.activation(
                        out=zz[:, FIN_W : 2 * FIN_W],
                        in_=tmp[:, FIN_W : 2 * FIN_W],
                        func=mybir.ActivationFunctionType.Relu,
                        bias=b3hi_sb[:, k : k + 1], scale=1.0,
                    )
                    nc.sync.dma_start(
                        xT[:].rearrange("(w r) m -> r w m", w=2)[:, :, nsl],
                        zz[:].rearrange("p (w m) -> p w m", w=2),
                    )

            # ---- final linear + log_softmax (node-lane, strided groups) ----
            NG = WIN // 128
            xTg = xT[:].rearrange("f (p g) -> f p g", g=NG)
            for g in range(NG):
                ps = ppool.tile([128, 6], F32, space="PSUM",
                                name=f"psf_{g}", tag="ps")
                nc.tensor.matmul(
                    ps[:], lhsT=xTg[:, :, g : g + 1], rhs=Wf_sb[:],
                    start=True, stop=True,
                )
                logits = fpool.tile([128, 6], F32, name=f"lg_{g}", tag="lg")
                nc.vector.tensor_add(logits[:], ps[:], bfb_sb[:])
                m = fpool.tile([128, 1], F32, name=f"m_{g}", tag="m")
                nc.vector.tensor_reduce(
                    out=m[:], in_=logits[:],
                    axis=mybir.AxisListType.X, op=mybir.AluOpType.max,
                )
                negm = fpool.tile([128, 1], F32, name=f"nm_{g}", tag="nm")
                nc.vector.tensor_scalar_mul(negm[:], m[:], -1.0)
                e = fpool.tile([128, 6], F32, name=f"e_{g}", tag="e")
                s = fpool.tile([128, 1], F32, name=f"s_{g}", tag="s")
                nc.scalar.activation(
                    out=e[:], in_=logits[:],
                    func=mybir.ActivationFunctionType.Exp,
                    bias=negm[:], scale=1.0, accum_out=s[:],
                )
                ls = fpool.tile([128, 1], F32, name=f"ls_{g}", tag="ls")
                nc.scalar.activation(
                    out=ls[:], in_=s[:], func=mybir.ActivationFunctionType.Ln
                )
                shift = fpool.tile([128, 1], F32, name=f"sh_{g}", tag="sh")
                nc.vector.tensor_sub(shift[:], negm[:], ls[:])
                nc.vector.tensor_scalar_add(
                    out_sb[:, 6 * g : 6 * (g + 1)], logits[:], shift[:]
                )

            nc.sync.dma_start(
                out_dram.ap().rearrange("(p g) f -> p (g f)", g=NG),
                out_sb[:],
            )

    nc.compile()
    return nc


def kernel(x, edge_index, W1, b1, W2, b2, W3, b3, Wf, bf):
    x = np.asarray(x, dtype=np.float32)
    n_nodes = x.shape[0]
    pl = _build_plan(np.asarray(edge_index), n_nodes)
    nc = _build_kernel(pl)
    in_maps = _make_in_maps(pl, x, W1, b1, W2, b2, W3, b3, Wf, bf)

    res = run_bass_kernel_spmd(nc, in_maps, core_ids=list(range(pl.n_cores)))

    LAST_RUN_INFO.clear()
    LAST_RUN_INFO["exec_time_ns"] = res.exec_time_ns
    LAST_RUN_INFO["mean_exec_time_ns"] = res.mean_exec_time_ns

    outs = [res.results[c]["out"] for c in range(pl.n_cores)]
    return _assemble_output(pl, outs, d_out=6)



# revision 16
# speedup vs baseline: 1.5178x; 1.5178x over previous
"""Trainium2 Bass kernel for 3-layer GCN + Linear + log_softmax.

nn_GCN3_Lin1: x[100000,16], edge_index[2,6400000] ->
  h = relu(gcn(x;W1,b1)); h = relu(gcn(h;W2,b2)); h = relu(gcn(h;W3,b3))
  out = log_softmax(h @ Wf + bf)

Strategy (8 NeuronCores, graph/data parallel over nodes):
  v2: Q7 does ONLY the gather (ap_gather); the scatter/segment-sum runs on
  the Tensor engine as one-hot indicator matmuls into per-window PSUM.

  - 100352 padded nodes; core c owns blocks [12544j+1568c, +1568) of every
    window j.  Band j of the SBUF node table (partitions [16j,16j+16)) holds
    core j's nodes as bf16 pairs [16, 12545, 2] (row 12544 = zero pad row).
  - Per layer: feature-lane GEMM -> u = dinv*h bf16 pair shard -> AllGather
    -> SBUF table.  Per (core, band): edges sorted by local dst, grouped
    R=2 per dst (zero-row padded), each 128-dst window padded to a fixed
    Gw groups so the SPMD program is data-independent.
  - ap_gather fetches the stream (call = 4 windows); DVE 2:1 pre-reduces
    pairs; per 128-group chunk: 2 tensor-engine transposes produce the
    edge-major rhs, a DVE is_equal(iota, dstloc) builds the one-hot lhsT,
    and a matmul accumulates into the window's [128 dst, 32] f32 PSUM.
  - Self-loop: 2 transposes of the own-shard table straight into PSUM
    (start=True).  Flush: z = relu(dinv*(S+u_own) + b) node-major, then one
    transpose back into the feature-major xT for the next layer's GEMM.
  - Final linear + log_softmax node-major as before.
"""

import numpy as np
from ml_dtypes import bfloat16

from concourse import mybir, bacc, tile
from concourse.bass_utils import run_bass_kernel_spmd

F32 = mybir.dt.float32
BF16 = mybir.dt.bfloat16
I16 = mybir.dt.int16

N_CORES = 8
WIN = 12544          # own nodes per core (= one table band)
BLK = 1568           # block of own nodes per (core, window)
NPAD = WIN * 8       # 100352
ZROW = WIN           # zero pad row in the table
PACK_W = 448         # nodes per pack chunk (28 chunks)
NPACK = WIN // PACK_W
R = 2                # edges pre-reduced per group (DVE)
NWIN = WIN // 128    # 98 dst windows per core
WQ = 4               # windows per full gather call
GW_MIN = 608         # min groups per (band, window); WQ*GW % 128 == 0

LAST_RUN_INFO = {}


class _Plan:
    pass


def _owner(g):
    return (g % WIN) // BLK


def _ownloc(g):
    return BLK * (g // WIN) + (g % BLK)


def _call_layout(gw, nwin):
    """Static chunk->window op pattern for a call covering `nwin` windows.

    Groups [w*gw, (w+1)*gw) belong to window w; chunks are 128-group
    slices (padded to whole chunks).  Returns (n_chunks, ops) with
    ops = [(chunk, lw, first_touch_of_chunk)] in window-major order.
    """
    n_groups = nwin * gw
    n_chunks = -(-n_groups // 128)
    ops = []
    seen = set()
    for lw in range(nwin):
        lo, hi = lw * gw, (lw + 1) * gw
        k0, k1 = lo // 128, min((hi - 1) // 128, n_chunks - 1)
        for k in range(k0, k1 + 1):
            first = k not in seen
            seen.add(k)
            ops.append((k, lw, first))
    return n_chunks, ops


def _build_plan(edge_index, n_nodes, n_cores=N_CORES):
    src = np.asarray(edge_index[0], dtype=np.int64)
    dst = np.asarray(edge_index[1], dtype=np.int64)
    assert n_nodes <= NPAD

    deg = np.bincount(dst, minlength=NPAD).astype(np.float64) + 1.0
    dinv = (1.0 / np.sqrt(deg)).astype(np.float32)

    oc = _owner(dst)
    band = _owner(src)
    ap_i = _ownloc(src)
    d_i = _ownloc(dst)

    streams = [[None] * 8 for _ in range(n_cores)]
    gcnt_all = np.zeros((n_cores, 8, WIN), np.int64)
    for c in range(n_cores):
        mc = oc == c
        b_c, a_c, d_c = band[mc], ap_i[mc], d_i[mc]
        for j in range(8):
            mj = b_c == j
            a, d = a_c[mj], d_c[mj]
            o = np.argsort(d, kind="stable")
            streams[c][j] = (d[o], a[o])
            gcnt_all[c, j] = -(-np.bincount(d, minlength=WIN) // R)
    wcnt = gcnt_all.reshape(n_cores, 8, NWIN, 128).sum(axis=3)
    gw = max(GW_MIN, (-(-int(wcnt.max()) // 32)) * 32)

    ncall = -(-NWIN // WQ)
    nwin_last = NWIN - WQ * (ncall - 1)
    n_chunks_f, ops_f = _call_layout(gw, WQ)
    n_chunks_l, ops_l = _call_layout(gw, nwin_last)
    gch = n_chunks_f * 128 * R
    gch_last = n_chunks_l * 128 * R
    tot_idx = (ncall - 1) * gch + gch_last
    full_gslots = n_chunks_f * 128
    ops_total = (ncall - 1) * 8 * len(ops_f) + 8 * len(ops_l)

    apidx = np.full((n_cores, 8, tot_idx), ZROW, np.int16)
    # dstloc[core, op, p] emitted in device order: t-major, band, op-in-call
    dstloc = np.full((n_cores, ops_total, 128), -1, np.int16)

    for c in range(n_cores):
        for j in range(8):
            d, a = streams[c][j]
            gcnt = gcnt_all[c, j]
            cnt = np.bincount(d, minlength=WIN)
            # edge slot array: per dst, R*gcnt slots, pads point at ZROW
            off_e = np.zeros(WIN + 1, np.int64)
            off_e[1:] = np.cumsum(R * gcnt)
            starts = np.searchsorted(d, np.arange(WIN))
            runpos = np.arange(len(d)) - starts[d]
            epad = np.full(off_e[-1], ZROW, np.int64)
            epad[off_e[d] + runpos] = a
            # groups in dst order
            gdst = np.repeat(np.arange(WIN), gcnt)
            gocc = (
                np.concatenate([np.arange(n) for n in gcnt if n > 0])
                if gdst.size
                else np.zeros(0, np.int64)
            )
            g_first_edge = off_e[gdst] + R * gocc
            # stream slot of each group (window region padded to gw)
            gwin = gdst // 128
            woff = np.zeros(NWIN + 1, np.int64)
            woff[1:] = np.cumsum(wcnt[c, j])
            g_window_pos = np.arange(len(gdst)) - woff[gwin]
            call_of_w = np.minimum(gwin // WQ, ncall - 1)
            lw = gwin - call_of_w * WQ
            slot = call_of_w * full_gslots + lw * gw + g_window_pos
            for m in range(R):
                apidx[c, j, slot * R + m] = epad[g_first_edge + m]
            # dstloc per op (device order: t, band, op)
            gmap_full = np.full((ncall - 1) * full_gslots + n_chunks_l * 128,
                                -1, np.int64)
            gmap_full[slot] = gdst
            op_base = 0
            for t in range(ncall):
                ops_t = ops_f if t < ncall - 1 else ops_l
                gslot0 = t * full_gslots
                for oi, (k, lwv, _first) in enumerate(ops_t):
                    w_global = t * WQ + lwv
                    gs = gmap_full[gslot0 + k * 128 : gslot0 + (k + 1) * 128]
                    dl = gs - 128 * w_global
                    dl[(gs < 0) | (dl < 0) | (dl >= 128)] = -1
                    # device op order: t, op-in-call, band-pair, half
                    col = op_base + oi * 8 + (j // 2) * 2 + (j % 2)
                    dstloc[c, col] = dl.astype(np.int16)
                op_base += 8 * len(ops_t)

    def wrap(arr):
        L = arr.shape[2]
        return np.ascontiguousarray(
            arr.reshape(n_cores, 8, L // 16, 16).transpose(0, 1, 3, 2).reshape(
                n_cores, 128, L // 16
            )
        )

    pl = _Plan()
    pl.n_cores = n_cores
    pl.n_nodes = n_nodes
    pl.gw = gw
    pl.ncall = ncall
    pl.gch = gch
    pl.gch_last = gch_last
    pl.n_chunks_f = n_chunks_f
    pl.n_chunks_l = n_chunks_l
    pl.ops_f = ops_f
    pl.ops_l = ops_l
    pl.ops_total = ops_total
    pl.apidx = wrap(apidx)
    # [cores, 128, ops_total] bf16 (values are small ints / -1, exact)
    pl.dstloc = np.ascontiguousarray(
        dstloc.transpose(0, 2, 1).astype(np.float32)
    )
    pl.dinv = dinv
    return pl


def _make_in_maps(pl, x, W1, b1, W2, b2, W3, b3, Wf, bf):
    n = x.shape[0]
    xpad = np.zeros((NPAD, 16), np.float32)
    xpad[:n] = np.asarray(x, np.float32)

    jj = np.arange(WIN) // BLK
    mm = np.arange(WIN) % BLK

    def wpad(W):
        a, b_ = W.shape
        out = np.zeros((32, 32), np.float32)
        out[:a, :b_] = W
        return out.astype(bfloat16)

    Wfp = np.zeros((32, 6), np.float32)
    Wfp[:12] = np.asarray(Wf, np.float32)
    Wfp = Wfp.astype(bfloat16)

    def bpad(b_):
        out = np.zeros(32, np.float32)
        out[: len(b_)] = b_
        return out

    biasb = np.zeros((128, 96), np.float32)
    for k, b_ in enumerate((b1, b2, b3)):
        biasb[:, 32 * k : 32 * k + 32] = bpad(np.asarray(b_, np.float32))[
            None, :
        ]
    bfb = np.tile(np.asarray(bf, np.float32)[None, :], (128, 1))

    iota128 = np.tile(
        np.arange(128, dtype=np.float32)[None, :], (128, 1)
    ).astype(bfloat16)
    ident16 = np.eye(16, dtype=np.float32).astype(bfloat16)
    ident32 = np.tile(np.eye(32, dtype=np.float32), (4, 1)).astype(bfloat16)
    ident128 = np.eye(128, dtype=np.float32).astype(bfloat16)

    in_maps = []
    for c in range(pl.n_cores):
        g = WIN * jj + BLK * c + mm
        xT = np.zeros((32, WIN), np.float32)
        xT[:16] = xpad[g].T
        d16 = np.tile(pl.dinv[g][None, :], (16, 1)).astype(bfloat16)
        dinvw = np.ascontiguousarray(
            pl.dinv[g].reshape(NWIN, 128).T.astype(np.float32)
        )
        in_maps.append(
            {
                "xT_in": xT.astype(bfloat16),
                "dinv_in": np.ascontiguousarray(d16),
                "dinvw_in": dinvw,
                "apidx_in": pl.apidx[c],
                "dstloc_in": pl.dstloc[c],
                "W1": wpad(W1),
                "W2": wpad(W2),
                "W3": wpad(W3),
                "Wf": np.ascontiguousarray(Wfp),
                "biasb": biasb,
                "bfb": bfb,
                "iota128": iota128,
                "ident16": ident16,
                "ident32": ident32,
                "ident128": ident128,
            }
        )
    return in_maps


def _assemble_output(pl, outs_per_core, d_out):
    full = np.empty((pl.n_nodes, d_out), dtype=np.float32)
    jj = np.arange(WIN) // BLK
    mm = np.arange(WIN) % BLK
    for c in range(pl.n_cores):
        gg = WIN * jj + BLK * c + mm
        keep = gg < pl.n_nodes
        full[gg[keep]] = outs_per_core[c][keep]
    return full


# ---------------------------------------------------------------------------
# Device kernel
# ---------------------------------------------------------------------------
def _build_kernel(pl):
    NC = pl.n_cores
    NCALL = pl.ncall
    GCH = pl.gch
    GCHL = pl.gch_last
    APCOL = ((NCALL - 1) * GCH + GCHL) // 16
    OPS = pl.ops_total

    nc = bacc.Bacc("TRN2", target_bir_lowering=False, debug=False,
                   num_devices=NC)

    xT_in = nc.dram_tensor("xT_in", [32, WIN], BF16, kind="ExternalInput")
    dinv_in = nc.dram_tensor("dinv_in", [16, WIN], BF16, kind="ExternalInput")
    dinvw_in = nc.dram_tensor("dinvw_in", [128, NWIN], F32,
                              kind="ExternalInput")
    apidx_in = nc.dram_tensor("apidx_in", [128, APCOL], I16,
                              kind="ExternalInput")
    dstloc_in = nc.dram_tensor("dstloc_in", [128, OPS], F32,
                               kind="ExternalInput")
    Ws = {
        n: nc.dram_tensor(n, [32, 32], BF16, kind="ExternalInput")
        for n in ("W1", "W2", "W3")
    }
    Wf_in = nc.dram_tensor("Wf", [32, 6], BF16, kind="ExternalInput")
    biasb_in = nc.dram_tensor("biasb", [128, 96], F32, kind="ExternalInput")
    bfb_in = nc.dram_tensor("bfb", [128, 6], F32, kind="ExternalInput")
    iota_in = nc.dram_tensor("iota128", [128, 128], BF16,
                             kind="ExternalInput")
    id16_in = nc.dram_tensor("ident16", [16, 16], BF16, kind="ExternalInput")
    id32_in = nc.dram_tensor("ident32", [128, 32], BF16, kind="ExternalInput")
    id128_in = nc.dram_tensor("ident128", [128, 128], BF16,
                              kind="ExternalInput")
    out_dram = nc.dram_tensor("out", [WIN, 6], F32, kind="ExternalOutput")

    shard = nc.dram_tensor("shard", [16, WIN * 2], BF16)
    table = nc.dram_tensor("table", [NC * 16, WIN * 2], BF16,
                           addr_space="Shared")
    rgroups = [list(range(NC))]

    with tile.TileContext(nc, num_cores=NC) as tc:
        with (
            tc.tile_pool(name="persist", bufs=1) as pers,
            tc.tile_pool(name="msg", bufs=2) as mpool,
            tc.tile_pool(name="gsum", bufs=2) as gpool,
            tc.tile_pool(name="idx", bufs=3) as ipool,
            tc.tile_pool(name="work", bufs=2) as wpool,
            tc.tile_pool(name="rhs", bufs=10) as rpool,
            tc.tile_pool(name="ind", bufs=8) as npool,
            tc.tile_pool(name="fin", bufs=3) as fpool,
            tc.tile_pool(name="ps", bufs=2, space="PSUM") as ppool,
            tc.tile_pool(name="pswin", bufs=2, space="PSUM") as wppool,
            tc.tile_pool(name="pstp", bufs=2, space="PSUM") as tppool,
            tc.tile_pool(name="psz", bufs=1, space="PSUM") as zppool,
        ):
            xT = pers.tile([32, WIN], BF16)
            nc.sync.dma_start(xT[:], xT_in[:, :])
            dinv16 = pers.tile([16, WIN], BF16)
            nc.sync.dma_start(dinv16[:], dinv_in[:, :])
            dinvw = pers.tile([128, NWIN], F32)
            nc.sync.dma_start(dinvw[:], dinvw_in[:, :])
            W_sb = {}
            for n in ("W1", "W2", "W3"):
                W_sb[n] = pers.tile([32, 32], BF16, name=f"Wsb_{n}")
                nc.sync.dma_start(W_sb[n][:], Ws[n][:, :])
            Wf_sb = pers.tile([32, 6], BF16)
            nc.sync.dma_start(Wf_sb[:], Wf_in[:, :])
            biasb_sb = pers.tile([128, 96], F32)
            nc.sync.dma_start(biasb_sb[:], biasb_in[:, :])
            bfb_sb = pers.tile([128, 6], F32)
            nc.sync.dma_start(bfb_sb[:], bfb_in[:, :])
            iota_sb = pers.tile([128, 128], BF16)
            nc.sync.dma_start(iota_sb[:], iota_in[:, :])
            id16_sb = pers.tile([16, 16], BF16)
            nc.sync.dma_start(id16_sb[:], id16_in[:, :])
            id32_sb = pers.tile([128, 32], BF16)
            nc.sync.dma_start(id32_sb[:], id32_in[:, :])
            id128_sb = pers.tile([128, 128], BF16)
            nc.sync.dma_start(id128_sb[:], id128_in[:, :])
            dstloc_sb = pers.tile([128, OPS], F32)
            nc.sync.dma_start(dstloc_sb[:], dstloc_in[:, :])

            tab = pers.tile([128, (WIN + 1) * 2], BF16)
            nc.vector.memset(tab[:, WIN * 2 : (WIN + 1) * 2], 0.0)
            out_sb = pers.tile([128, (WIN // 128) * 6], F32)

            for k, wname in enumerate(("W1", "W2", "W3")):
                op_idx = 0
                # ---- pack: u = dinv * (x @ W), bf16 pair-packed shard ----
                for t in range(NPACK):
                    sl = slice(PACK_W * t, PACK_W * (t + 1))
                    ps_lo = ppool.tile([16, PACK_W], F32, space="PSUM",
                                       name=f"pl_{k}_{t}", tag="ps")
                    nc.tensor.matmul(
                        ps_lo[:], lhsT=W_sb[wname][:, 0:16], rhs=xT[:, sl],
                        start=True, stop=True,
                    )
                    ps_hi = ppool.tile([16, PACK_W], F32, space="PSUM",
                                       name=f"ph_{k}_{t}", tag="ps")
                    nc.tensor.matmul(
                        ps_hi[:], lhsT=W_sb[wname][:, 16:32], rhs=xT[:, sl],
                        start=True, stop=True,
                    )
                    dvf = wpool.tile([16, PACK_W], F32,
                                     name=f"dv_{k}_{t}", tag="dv")
                    nc.vector.tensor_copy(dvf[:], dinv16[:, sl])
                    pk = wpool.tile([16, PACK_W * 2], BF16,
                                    name=f"pk_{k}_{t}", tag="pk")
                    pkv = pk[:].rearrange("p (m w) -> p m w", w=2)
                    nc.vector.tensor_mul(pkv[:, :, 0:1], ps_lo[:], dvf[:])
                    nc.vector.tensor_mul(pkv[:, :, 1:2], ps_hi[:], dvf[:])
                    nc.sync.dma_start(
                        shard[:, PACK_W * 2 * t : PACK_W * 2 * (t + 1)], pk[:]
                    )
                nc.gpsimd.collective_compute(
                    "AllGather",
                    mybir.AluOpType.bypass,
                    replica_groups=rgroups,
                    ins=[shard.ap().opt()],
                    outs=[table.ap().opt()],
                )
                nc.sync.dma_start(tab[:, : WIN * 2], table[:, :])

                # ---- per call: gather + R:1 pre-reduce, then windows ----
                for t in range(NCALL):
                    g_t = GCH if t < NCALL - 1 else GCHL
                    ia = ipool.tile([128, GCH // 16], I16,
                                    name=f"ia_{k}_{t}", tag="ia")
                    nc.sync.dma_start(
                        ia[:, : g_t // 16],
                        apidx_in[:, (GCH // 16) * t :
                                 (GCH // 16) * t + g_t // 16],
                    )
                    mg = mpool.tile([128, GCH * 2], BF16,
                                    name=f"mg_{k}_{t}", tag="mg")
                    nc.gpsimd.ap_gather(
                        out_ap=mg[:].rearrange("p (i w) -> p i w", w=2)[
                            :, :g_t, :
                        ],
                        in_ap=tab[:].rearrange("p (e w) -> p e w", w=2),
                        idxs_ap=ia[:, : g_t // 16],
                        channels=128, num_elems=WIN + 1, d=2, num_idxs=g_t,
                    )
                    gs = gpool.tile([128, (GCH // R) * 2], BF16,
                                    name=f"gs_{k}_{t}", tag="gs")
                    with nc.allow_low_precision(reason="R-edge bf16 presums"):
                        nc.vector.tensor_reduce(
                            out=gs[:].rearrange("p (g w) -> p g w", w=2)[
                                :, : g_t // R, :
                            ],
                            in_=mg[:].rearrange(
                                "p (g e w) -> p g w e", e=R, w=2
                            )[:, : g_t // R, :, :],
                            axis=mybir.AxisListType.X,
                            op=mybir.AluOpType.add,
                        )

                    # ---- windows of this call ----
                    ops_t = pl.ops_f if t < NCALL - 1 else pl.ops_l
                    nwin_t = WQ if t < NCALL - 1 else NWIN - WQ * (NCALL - 1)
                    gsv = gs[:].rearrange("p (g w) -> p g w", w=2)
                    rhs_of_chunk = {}
                    ops_by_w = {}
                    for (ck2, l2, _f2) in ops_t:
                        ops_by_w.setdefault(l2, []).append(ck2)
                    for lw in range(nwin_t):
                        w = t * WQ + lw
                        ps = wppool.tile([128, 32], F32, space="PSUM",
                                         name=f"win_{k}_{w}", tag="win")
                        # self-loop: fetch own u window, transpose, feed as
                        # a chunk with identity indicator (starts the group)
                        ow = ipool.tile([16, 256], BF16,
                                        name=f"ow_{k}_{w}", tag="ow")
                        nc.sync.dma_start(
                            ow[:], shard[:, 256 * w : 256 * w + 256]
                        )
                        owv = ow[:].rearrange("p (e w) -> p e w", w=2)
                        tps = tppool.tile([128, 32], BF16, space="PSUM",
                                          name=f"tps_{k}_{w}", tag="tp")
                        for wsel in range(2):
                            nc.tensor.matmul(
                                tps[:, 16 * wsel : 16 * wsel + 16],
                                lhsT=owv[:, :, wsel],
                                rhs=id16_sb[:],
                                is_transpose=True, start=True,
                                stop=(wsel == 1),
                                skip_group_check=True,
                            )
                        rbs = rpool.tile([128, 32], BF16,
                                         name=f"rbs_{k}_{w}", tag="rb")
                        nc.vector.tensor_copy(rbs[:], tps[:])
                        nc.tensor.matmul(
                            ps[:], lhsT=id128_sb[:], rhs=rbs[:],
                            start=True, stop=False, skip_group_check=True,
                        )
                        n_ops_lw = 8 * len(ops_by_w[lw])
                        done = 0
                        for ck in ops_by_w[lw]:
                            for bp in range(4):
                                key = (bp, ck)
                                if key not in rhs_of_chunk:
                                    tp = tppool.tile(
                                        [128, 64], BF16, space="PSUM",
                                        name=f"tp_{k}_{t}_{bp}_{ck}",
                                        tag="tp",
                                    )
                                    for wsel in range(2):
                                        nc.tensor.matmul(
                                            tp[:, 32 * wsel : 32 * wsel + 32],
                                            lhsT=gsv[
                                                32 * bp : 32 * bp + 32,
                                                128 * ck : 128 * ck + 128,
                                                wsel,
                                            ],
                                            rhs=id32_sb[
                                                32 * bp : 32 * bp + 32, :
                                            ],
                                            is_transpose=True,
                                            start=True, stop=(wsel == 1),
                                            skip_group_check=True,
                                            tile_position=(32 * bp, 0),
                                        )
                                    # [wsel(2) x (half(2) x 16)] -> per-band
                                    tpv = tp[:].rearrange(
                                        "p (q h r) -> p h q r", q=2, h=2
                                    )
                                    rbs2 = []
                                    for h in range(2):
                                        rb = rpool.tile(
                                            [128, 32], BF16,
                                            name=f"rb_{k}_{t}_{bp}_{ck}_{h}",
                                            tag="rb",
                                        )
                                        nc.vector.tensor_copy(
                                            rb[:].rearrange(
                                                "p (q r) -> p q r", r=16
                                            ),
                                            tpv[:, h],
                                        )
                                        rbs2.append(rb)
                                    rhs_of_chunk[key] = rbs2
                                rbs2 = rhs_of_chunk[key]
                                for h in range(2):
                                    ind = npool.tile(
                                        [128, 128], BF16,
                                        name=f"in_{op_idx}", tag="in",
                                    )
                                    nc.vector.tensor_scalar(
                                        out=ind[:],
                                        in0=iota_sb[:],
                                        scalar1=dstloc_sb[
                                            :, op_idx : op_idx + 1
                                        ],
                                        scalar2=None,
                                        op0=mybir.AluOpType.is_equal,
                                    )
                                    done += 1
                                    nc.tensor.matmul(
                                        ps[:],
                                        lhsT=ind[:],
                                        rhs=rbs2[h][:],
                                        start=False,
                                        stop=done == n_ops_lw,
                                        skip_group_check=True,
                                    )
                                    op_idx += 1
                        # flush this window
                        t1 = fpool.tile([128, 32], F32, name=f"t1_{k}_{w}",
                                        tag="t1")
                        nc.vector.tensor_scalar_mul(
                            t1[:], ps[:], dinvw[:, w : w + 1]
                        )
                        t2 = fpool.tile([128, 32], F32, name=f"t2_{k}_{w}",
                                        tag="t2")
                        nc.vector.tensor_add(
                            t2[:], t1[:], biasb_sb[:, 32 * k : 32 * k + 32]
                        )
                        zb = fpool.tile([128, 32], BF16, name=f"zb_{k}_{w}",
                                        tag="zb")
                        nc.vector.tensor_scalar_max(zb[:], t2[:], 0.0)
                        zp = zppool.tile([32, 128], BF16, space="PSUM",
                                         name=f"zp_{k}_{w}", tag="zp")
                        nc.tensor.matmul(
                            zp[:], lhsT=zb[:], rhs=id128_sb[:],
                            is_transpose=True, start=True, stop=True,
                            skip_group_check=True,
                        )
                        nc.vector.tensor_copy(
                            xT[:, 128 * w : 128 * w + 128], zp[:]
                        )

            # ---- final linear + log_softmax (node-lane, strided groups) ----
            NG = WIN // 128
            xTg = xT[:].rearrange("f (p g) -> f p g", g=NG)
            for g in range(NG):
                ps = ppool.tile([128, 6], F32, space="PSUM",
                                name=f"psf_{g}", tag="ps")
                nc.tensor.matmul(
                    ps[:], lhsT=xTg[:, :, g : g + 1], rhs=Wf_sb[:],
                    start=True, stop=True,
                )
                logits = fpool.tile([128, 6], F32, name=f"lg_{g}", tag="lg")
                nc.vector.tensor_add(logits[:], ps[:], bfb_sb[:])
                m = fpool.tile([128, 1], F32, name=f"m_{g}", tag="m")
                nc.vector.tensor_reduce(
                    out=m[:], in_=logits[:],
                    axis=mybir.AxisListType.X, op=mybir.AluOpType.max,
                )
                negm = fpool.tile([128, 1], F32, name=f"nm_{g}", tag="nm")
                nc.vector.tensor_scalar_mul(negm[:], m[:], -1.0)
                e = fpool.tile([128, 6], F32, name=f"e_{g}", tag="e")
                s = fpool.tile([128, 1], F32, name=f"s_{g}", tag="s")
                nc.scalar.activation(
                    out=e[:], in_=logits[:],
                    func=mybir.ActivationFunctionType.Exp,
                    bias=negm[:], scale=1.0, accum_out=s[:],
                )
                ls = fpool.tile([128, 1], F32, name=f"ls_{g}", tag="ls")
                nc.scalar.activation(
                    out=ls[:], in_=s[:], func=mybir.ActivationFunctionType.Ln
                )
                shift = fpool.tile([128, 1], F32, name=f"sh_{g}", tag="sh")
                nc.vector.tensor_sub(shift[:], negm[:], ls[:])
                nc.vector.tensor_scalar_add(
                    out_sb[:, 6 * g : 6 * (g + 1)], logits[:], shift[:]
                )

            nc.sync.dma_start(
                out_dram.ap().rearrange("(p g) f -> p (g f)", g=NG),
                out_sb[:],
            )

    nc.compile()
    return nc


def kernel(x, edge_index, W1, b1, W2, b2, W3, b3, Wf, bf):
    x = np.asarray(x, dtype=np.float32)
    n_nodes = x.shape[0]
    pl = _build_plan(np.asarray(edge_index), n_nodes)
    nc = _build_kernel(pl)
    in_maps = _make_in_maps(pl, x, W1, b1, W2, b2, W3, b3, Wf, bf)

    res = run_bass_kernel_spmd(nc, in_maps, core_ids=list(range(pl.n_cores)))

    LAST_RUN_INFO.clear()
    LAST_RUN_INFO["exec_time_ns"] = res.exec_time_ns
    LAST_RUN_INFO["mean_exec_time_ns"] = res.mean_exec_time_ns

    outs = [res.results[c]["out"] for c in range(pl.n_cores)]
    return _assemble_output(pl, outs, d_out=6)


# revision 18
# speedup vs baseline: 1.5913x; 1.0485x over previous
"""Trainium2 Bass kernel for 3-layer GCN + Linear + log_softmax.

nn_GCN3_Lin1: x[100000,16], edge_index[2,6400000] ->
  h = relu(gcn(x;W1,b1)); h = relu(gcn(h;W2,b2)); h = relu(gcn(h;W3,b3))
  out = log_softmax(h @ Wf + bf)

Strategy (8 NeuronCores, graph/data parallel over nodes):
  v2: Q7 does ONLY the gather (ap_gather); the scatter/segment-sum runs on
  the Tensor engine as one-hot indicator matmuls into per-window PSUM.

  - 100352 padded nodes; core c owns blocks [12544j+1568c, +1568) of every
    window j.  Band j of the SBUF node table (partitions [16j,16j+16)) holds
    core j's nodes as bf16 pairs [16, 12545, 2] (row 12544 = zero pad row).
  - Per layer: feature-lane GEMM -> u = dinv*h bf16 pair shard -> AllGather
    -> SBUF table.  Per (core, band): edges sorted by local dst, grouped
    R=2 per dst (zero-row padded), each 128-dst window padded to a fixed
    Gw groups so the SPMD program is data-independent.
  - ap_gather fetches the stream (call = 4 windows); DVE 2:1 pre-reduces
    pairs; per 128-group chunk: 2 tensor-engine transposes produce the
    edge-major rhs, a DVE is_equal(iota, dstloc) builds the one-hot lhsT,
    and a matmul accumulates into the window's [128 dst, 32] f32 PSUM.
  - Self-loop: 2 transposes of the own-shard table straight into PSUM
    (start=True).  Flush: z = relu(dinv*(S+u_own) + b) node-major, then one
    transpose back into the feature-major xT for the next layer's GEMM.
  - Final linear + log_softmax node-major as before.
"""

import numpy as np
from ml_dtypes import bfloat16

from concourse import mybir, bacc, tile
from concourse.bass_utils import run_bass_kernel_spmd

F32 = mybir.dt.float32
BF16 = mybir.dt.bfloat16
I16 = mybir.dt.int16

N_CORES = 8
WIN = 12544          # own nodes per core (= one table band)
BLK = 1568           # block of own nodes per (core, window)
NPAD = WIN * 8       # 100352
ZROW = WIN           # zero pad row in the table
PACK_W = 448         # nodes per pack chunk (28 chunks)
NPACK = WIN // PACK_W
R = 2                # edges pre-reduced per group (DVE)
NWIN = WIN // 128    # 98 dst windows per core
WQ = 4               # windows per full gather call
GW_MIN = 320         # min groups per (band, window); WQ*GW % 128 == 0

LAST_RUN_INFO = {}


class _Plan:
    pass


def _owner(g):
    return (g % WIN) // BLK


def _ownloc(g):
    return BLK * (g // WIN) + (g % BLK)


def _call_layout(gw, nwin):
    """Static chunk->window op pattern for a call covering `nwin` windows.

    Groups [w*gw, (w+1)*gw) belong to window w; chunks are 128-group
    slices (padded to whole chunks).  Returns (n_chunks, ops) with
    ops = [(chunk, lw, first_touch_of_chunk)] in window-major order.
    """
    n_groups = nwin * gw
    n_chunks = -(-n_groups // 128)
    ops = []
    seen = set()
    for lw in range(nwin):
        lo, hi = lw * gw, (lw + 1) * gw
        k0, k1 = lo // 128, min((hi - 1) // 128, n_chunks - 1)
        for k in range(k0, k1 + 1):
            first = k not in seen
            seen.add(k)
            ops.append((k, lw, first))
    return n_chunks, ops


def _build_plan(edge_index, n_nodes, n_cores=N_CORES):
    src = np.asarray(edge_index[0], dtype=np.int64)
    dst = np.asarray(edge_index[1], dtype=np.int64)
    assert n_nodes <= NPAD

    deg = np.bincount(dst, minlength=NPAD).astype(np.float64) + 1.0
    dinv = (1.0 / np.sqrt(deg)).astype(np.float32)

    oc = _owner(dst)
    band = _owner(src)
    ap_i = _ownloc(src)
    d_i = _ownloc(dst)

    gcnt_raw = np.zeros((n_cores, 8, WIN), np.int64)
    for c in range(n_cores):
        mc = oc == c
        b_c, d_c = band[mc], d_i[mc]
        for j in range(8):
            dd = d_c[b_c == j]
            gcnt_raw[c, j] = -(-np.bincount(dd, minlength=WIN) // R)

    # balance dst->window assignment: per core, greedily permute local slots
    # so every (band, window) group count is near the mean.
    invperm = np.empty((n_cores, WIN), np.int64)  # old slot -> new slot
    glob = np.empty((n_cores, WIN), np.int64)     # new slot -> global node
    jj0 = np.arange(WIN) // BLK
    mm0 = np.arange(WIN) % BLK
    for c in range(n_cores):
        v = gcnt_raw[c]                         # [8, WIN]
        order_sorted = np.argsort(-v.sum(0), kind="stable")
        Wsum = np.zeros((NWIN, 8), np.int64)
        ncount = np.zeros(NWIN, np.int64)
        assign = np.empty(WIN, np.int64)
        for s in order_sorted:
            cand = (Wsum + v[:, s][None, :]).max(axis=1)
            cand[ncount >= 128] = 1 << 40
            w = int(np.argmin(cand))
            assign[s] = w
            Wsum[w] += v[:, s]
            ncount[w] += 1
        order = np.argsort(assign * WIN + np.arange(WIN), kind="stable")
        invperm[c, order] = np.arange(WIN)
        glob[c] = (WIN * jj0 + BLK * c + mm0)[order]

    # re-map edge endpoints to permuted local slots
    ap_i = invperm[band, ap_i]
    d_i = invperm[oc, d_i]

    streams = [[None] * 8 for _ in range(n_cores)]
    gcnt_all = np.zeros((n_cores, 8, WIN), np.int64)
    for c in range(n_cores):
        mc = oc == c
        b_c, a_c, d_c = band[mc], ap_i[mc], d_i[mc]
        for j in range(8):
            mj = b_c == j
            a, d = a_c[mj], d_c[mj]
            o = np.argsort(d, kind="stable")
            streams[c][j] = (d[o], a[o])
            gcnt_all[c, j] = -(-np.bincount(d, minlength=WIN) // R)
    wcnt = gcnt_all.reshape(n_cores, 8, NWIN, 128).sum(axis=3)
    gw = max(GW_MIN, (-(-int(wcnt.max()) // 32)) * 32)

    ncall = -(-NWIN // WQ)
    nwin_last = NWIN - WQ * (ncall - 1)
    n_chunks_f, ops_f = _call_layout(gw, WQ)
    n_chunks_l, ops_l = _call_layout(gw, nwin_last)
    gch = n_chunks_f * 128 * R
    gch_last = n_chunks_l * 128 * R
    tot_idx = (ncall - 1) * gch + gch_last
    full_gslots = n_chunks_f * 128
    ops_total = (ncall - 1) * 8 * len(ops_f) + 8 * len(ops_l)

    apidx = np.full((n_cores, 8, tot_idx), ZROW, np.int16)
    # dstloc[core, op, p] emitted in device order: t-major, band, op-in-call
    dstloc = np.full((n_cores, ops_total, 128), -1, np.int16)

    for c in range(n_cores):
        for j in range(8):
            d, a = streams[c][j]
            gcnt = gcnt_all[c, j]
            cnt = np.bincount(d, minlength=WIN)
            # edge slot array: per dst, R*gcnt slots, pads point at ZROW
            off_e = np.zeros(WIN + 1, np.int64)
            off_e[1:] = np.cumsum(R * gcnt)
            starts = np.searchsorted(d, np.arange(WIN))
            runpos = np.arange(len(d)) - starts[d]
            epad = np.full(off_e[-1], ZROW, np.int64)
            epad[off_e[d] + runpos] = a
            # groups in dst order
            gdst = np.repeat(np.arange(WIN), gcnt)
            gocc = (
                np.concatenate([np.arange(n) for n in gcnt if n > 0])
                if gdst.size
                else np.zeros(0, np.int64)
            )
            g_first_edge = off_e[gdst] + R * gocc
            # stream slot of each group (window region padded to gw)
            gwin = gdst // 128
            woff = np.zeros(NWIN + 1, np.int64)
            woff[1:] = np.cumsum(wcnt[c, j])
            g_window_pos = np.arange(len(gdst)) - woff[gwin]
            call_of_w = np.minimum(gwin // WQ, ncall - 1)
            lw = gwin - call_of_w * WQ
            slot = call_of_w * full_gslots + lw * gw + g_window_pos
            for m in range(R):
                apidx[c, j, slot * R + m] = epad[g_first_edge + m]
            # dstloc per op (device order: t, band, op)
            gmap_full = np.full((ncall - 1) * full_gslots + n_chunks_l * 128,
                                -1, np.int64)
            gmap_full[slot] = gdst
            op_base = 0
            for t in range(ncall):
                ops_t = ops_f if t < ncall - 1 else ops_l
                gslot0 = t * full_gslots
                for oi, (k, lwv, _first) in enumerate(ops_t):
                    w_global = t * WQ + lwv
                    gs = gmap_full[gslot0 + k * 128 : gslot0 + (k + 1) * 128]
                    dl = gs - 128 * w_global
                    dl[(gs < 0) | (dl < 0) | (dl >= 128)] = -1
                    # device op order: t, op-in-call, band-pair, half
                    col = op_base + oi * 8 + (j // 2) * 2 + (j % 2)
                    dstloc[c, col] = dl.astype(np.int16)
                op_base += 8 * len(ops_t)

    def wrap(arr):
        L = arr.shape[2]
        return np.ascontiguousarray(
            arr.reshape(n_cores, 8, L // 16, 16).transpose(0, 1, 3, 2).reshape(
                n_cores, 128, L // 16
            )
        )

    pl = _Plan()
    pl.n_cores = n_cores
    pl.n_nodes = n_nodes
    pl.gw = gw
    pl.ncall = ncall
    pl.gch = gch
    pl.gch_last = gch_last
    pl.n_chunks_f = n_chunks_f
    pl.n_chunks_l = n_chunks_l
    pl.ops_f = ops_f
    pl.ops_l = ops_l
    pl.ops_total = ops_total
    pl.apidx = wrap(apidx)
    pl.glob = glob
    # [cores, 128, ops_total] bf16 (values are small ints / -1, exact)
    pl.dstloc = np.ascontiguousarray(
        dstloc.transpose(0, 2, 1).astype(np.float32)
    )
    pl.dinv = dinv
    return pl


def _make_in_maps(pl, x, W1, b1, W2, b2, W3, b3, Wf, bf):
    n = x.shape[0]
    xpad = np.zeros((NPAD, 16), np.float32)
    xpad[:n] = np.asarray(x, np.float32)

    def wpad(W):
        a, b_ = W.shape
        out = np.zeros((32, 32), np.float32)
        out[:a, :b_] = W
        return out.astype(bfloat16)

    Wfp = np.zeros((32, 6), np.float32)
    Wfp[:12] = np.asarray(Wf, np.float32)
    Wfp = Wfp.astype(bfloat16)

    def bpad(b_):
        out = np.zeros(32, np.float32)
        out[: len(b_)] = b_
        return out

    biasb = np.zeros((128, 96), np.float32)
    for k, b_ in enumerate((b1, b2, b3)):
        biasb[:, 32 * k : 32 * k + 32] = bpad(np.asarray(b_, np.float32))[
            None, :
        ]
    bfb = np.tile(np.asarray(bf, np.float32)[None, :], (128, 1))

    iota128 = np.tile(
        np.arange(128, dtype=np.float32)[None, :], (128, 1)
    ).astype(bfloat16)
    ident16 = np.eye(16, dtype=np.float32).astype(bfloat16)
    ident32 = np.tile(np.eye(32, dtype=np.float32), (4, 1)).astype(bfloat16)
    ident128 = np.eye(128, dtype=np.float32).astype(bfloat16)

    in_maps = []
    for c in range(pl.n_cores):
        g = pl.glob[c]
        xT = np.zeros((32, WIN), np.float32)
        xT[:16] = xpad[g].T
        d16 = np.tile(pl.dinv[g][None, :], (16, 1)).astype(bfloat16)
        dinvw = np.ascontiguousarray(
            pl.dinv[g].reshape(NWIN, 128).T.astype(np.float32)
        )
        in_maps.append(
            {
                "xT_in": xT.astype(bfloat16),
                "dinv_in": np.ascontiguousarray(d16),
                "dinvw_in": dinvw,
                "apidx_in": pl.apidx[c],
                "dstloc_in": pl.dstloc[c],
                "W1": wpad(W1),
                "W2": wpad(W2),
                "W3": wpad(W3),
                "Wf": np.ascontiguousarray(Wfp),
                "biasb": biasb,
                "bfb": bfb,
                "iota128": iota128,
                "ident16": ident16,
                "ident32": ident32,
                "ident128": ident128,
            }
        )
    return in_maps


def _assemble_output(pl, outs_per_core, d_out):
    full = np.empty((pl.n_nodes, d_out), dtype=np.float32)
    for c in range(pl.n_cores):
        gg = pl.glob[c]
        keep = gg < pl.n_nodes
        full[gg[keep]] = outs_per_core[c][keep]
    return full


# ---------------------------------------------------------------------------
# Device kernel
# ---------------------------------------------------------------------------
def _build_kernel(pl):
    NC = pl.n_cores
    NCALL = pl.ncall
    GCH = pl.gch
    GCHL = pl.gch_last
    APCOL = ((NCALL - 1) * GCH + GCHL) // 16
    OPS = pl.ops_total

    nc = bacc.Bacc("TRN2", target_bir_lowering=False, debug=False,
                   num_devices=NC)

    xT_in = nc.dram_tensor("xT_in", [32, WIN], BF16, kind="ExternalInput")
    dinv_in = nc.dram_tensor("dinv_in", [16, WIN], BF16, kind="ExternalInput")
    dinvw_in = nc.dram_tensor("dinvw_in", [128, NWIN], F32,
                              kind="ExternalInput")
    apidx_in = nc.dram_tensor("apidx_in", [128, APCOL], I16,
                              kind="ExternalInput")
    dstloc_in = nc.dram_tensor("dstloc_in", [128, OPS], F32,
                               kind="ExternalInput")
    Ws = {
        n: nc.dram_tensor(n, [32, 32], BF16, kind="ExternalInput")
        for n in ("W1", "W2", "W3")
    }
    Wf_in = nc.dram_tensor("Wf", [32, 6], BF16, kind="ExternalInput")
    biasb_in = nc.dram_tensor("biasb", [128, 96], F32, kind="ExternalInput")
    bfb_in = nc.dram_tensor("bfb", [128, 6], F32, kind="ExternalInput")
    iota_in = nc.dram_tensor("iota128", [128, 128], BF16,
                             kind="ExternalInput")
    id16_in = nc.dram_tensor("ident16", [16, 16], BF16, kind="ExternalInput")
    id32_in = nc.dram_tensor("ident32", [128, 32], BF16, kind="ExternalInput")
    id128_in = nc.dram_tensor("ident128", [128, 128], BF16,
                              kind="ExternalInput")
    out_dram = nc.dram_tensor("out", [WIN, 6], F32, kind="ExternalOutput")

    shard = nc.dram_tensor("shard", [16, WIN * 2], BF16)
    table = nc.dram_tensor("table", [NC * 16, WIN * 2], BF16,
                           addr_space="Shared")
    rgroups = [list(range(NC))]

    with tile.TileContext(nc, num_cores=NC) as tc:
        with (
            tc.tile_pool(name="persist", bufs=1) as pers,
            tc.tile_pool(name="msg", bufs=2) as mpool,
            tc.tile_pool(name="gsum", bufs=2) as gpool,
            tc.tile_pool(name="idx", bufs=3) as ipool,
            tc.tile_pool(name="work", bufs=2) as wpool,
            tc.tile_pool(name="rhs", bufs=10) as rpool,
            tc.tile_pool(name="ind", bufs=8) as npool,
            tc.tile_pool(name="fin", bufs=3) as fpool,
            tc.tile_pool(name="ps", bufs=2, space="PSUM") as ppool,
            tc.tile_pool(name="pswin", bufs=2, space="PSUM") as wppool,
            tc.tile_pool(name="pstp", bufs=2, space="PSUM") as tppool,
            tc.tile_pool(name="psz", bufs=1, space="PSUM") as zppool,
        ):
            xT = pers.tile([32, WIN], BF16)
            nc.sync.dma_start(xT[:], xT_in[:, :])
            dinv16 = pers.tile([16, WIN], BF16)
            nc.sync.dma_start(dinv16[:], dinv_in[:, :])
            dinvw = pers.tile([128, NWIN], F32)
            nc.sync.dma_start(dinvw[:], dinvw_in[:, :])
            W_sb = {}
            for n in ("W1", "W2", "W3"):
                W_sb[n] = pers.tile([32, 32], BF16, name=f"Wsb_{n}")
                nc.sync.dma_start(W_sb[n][:], Ws[n][:, :])
            Wf_sb = pers.tile([32, 6], BF16)
            nc.sync.dma_start(Wf_sb[:], Wf_in[:, :])
            biasb_sb = pers.tile([128, 96], F32)
            nc.sync.dma_start(biasb_sb[:], biasb_in[:, :])
            bfb_sb = pers.tile([128, 6], F32)
            nc.sync.dma_start(bfb_sb[:], bfb_in[:, :])
            iota_sb = pers.tile([128, 128], BF16)
            nc.sync.dma_start(iota_sb[:], iota_in[:, :])
            id16_sb = pers.tile([16, 16], BF16)
            nc.sync.dma_start(id16_sb[:], id16_in[:, :])
            id32_sb = pers.tile([128, 32], BF16)
            nc.sync.dma_start(id32_sb[:], id32_in[:, :])
            id128_sb = pers.tile([128, 128], BF16)
            nc.sync.dma_start(id128_sb[:], id128_in[:, :])
            dstloc_sb = pers.tile([128, OPS], F32)
            nc.sync.dma_start(dstloc_sb[:], dstloc_in[:, :])

            tab = pers.tile([128, (WIN + 1) * 2], BF16)
            nc.vector.memset(tab[:, WIN * 2 : (WIN + 1) * 2], 0.0)
            out_sb = pers.tile([128, (WIN // 128) * 6], F32)

            for k, wname in enumerate(("W1", "W2", "W3")):
                op_idx = 0
                # ---- pack: u = dinv * (x @ W), bf16 pair-packed shard ----
                for t in range(NPACK):
                    sl = slice(PACK_W * t, PACK_W * (t + 1))
                    ps_lo = ppool.tile([16, PACK_W], F32, space="PSUM",
                                       name=f"pl_{k}_{t}", tag="ps")
                    nc.tensor.matmul(
                        ps_lo[:], lhsT=W_sb[wname][:, 0:16], rhs=xT[:, sl],
                        start=True, stop=True,
                    )
                    ps_hi = ppool.tile([16, PACK_W], F32, space="PSUM",
                                       name=f"ph_{k}_{t}", tag="ps")
                    nc.tensor.matmul(
                        ps_hi[:], lhsT=W_sb[wname][:, 16:32], rhs=xT[:, sl],
                        start=True, stop=True,
                    )
                    dvf = wpool.tile([16, PACK_W], F32,
                                     name=f"dv_{k}_{t}", tag="dv")
                    nc.vector.tensor_copy(dvf[:], dinv16[:, sl])
                    pk = wpool.tile([16, PACK_W * 2], BF16,
                                    name=f"pk_{k}_{t}", tag="pk")
                    pkv = pk[:].rearrange("p (m w) -> p m w", w=2)
                    nc.vector.tensor_mul(pkv[:, :, 0:1], ps_lo[:], dvf[:])
                    nc.vector.tensor_mul(pkv[:, :, 1:2], ps_hi[:], dvf[:])
                    nc.sync.dma_start(
                        shard[:, PACK_W * 2 * t : PACK_W * 2 * (t + 1)], pk[:]
                    )
                nc.gpsimd.collective_compute(
                    "AllGather",
                    mybir.AluOpType.bypass,
                    replica_groups=rgroups,
                    ins=[shard.ap().opt()],
                    outs=[table.ap().opt()],
                )
                nc.sync.dma_start(tab[:, : WIN * 2], table[:, :])

                # ---- per call: gather + R:1 pre-reduce, then windows ----
                for t in range(NCALL):
                    g_t = GCH if t < NCALL - 1 else GCHL
                    ia = ipool.tile([128, GCH // 16], I16,
                                    name=f"ia_{k}_{t}", tag="ia")
                    nc.sync.dma_start(
                        ia[:, : g_t // 16],
                        apidx_in[:, (GCH // 16) * t :
                                 (GCH // 16) * t + g_t // 16],
                    )
                    mg = mpool.tile([128, GCH * 2], BF16,
                                    name=f"mg_{k}_{t}", tag="mg")
                    nc.gpsimd.ap_gather(
                        out_ap=mg[:].rearrange("p (i w) -> p i w", w=2)[
                            :, :g_t, :
                        ],
                        in_ap=tab[:].rearrange("p (e w) -> p e w", w=2),
                        idxs_ap=ia[:, : g_t // 16],
                        channels=128, num_elems=WIN + 1, d=2, num_idxs=g_t,
                    )
                    gs = gpool.tile([128, (GCH // R) * 2], BF16,
                                    name=f"gs_{k}_{t}", tag="gs")
                    with nc.allow_low_precision(reason="R-edge bf16 presums"):
                        nc.vector.tensor_reduce(
                            out=gs[:].rearrange("p (g w) -> p g w", w=2)[
                                :, : g_t // R, :
                            ],
                            in_=mg[:].rearrange(
                                "p (g e w) -> p g w e", e=R, w=2
                            )[:, : g_t // R, :, :],
                            axis=mybir.AxisListType.X,
                            op=mybir.AluOpType.add,
                        )

                    # ---- windows of this call ----
                    ops_t = pl.ops_f if t < NCALL - 1 else pl.ops_l
                    nwin_t = WQ if t < NCALL - 1 else NWIN - WQ * (NCALL - 1)
                    gsv = gs[:].rearrange("p (g w) -> p g w", w=2)
                    rhs_of_chunk = {}
                    ops_by_w = {}
                    for (ck2, l2, _f2) in ops_t:
                        ops_by_w.setdefault(l2, []).append(ck2)
                    for lw in range(nwin_t):
                        w = t * WQ + lw
                        ps = wppool.tile([128, 32], F32, space="PSUM",
                                         name=f"win_{k}_{w}", tag="win")
                        # self-loop: fetch own u window, transpose, feed as
                        # a chunk with identity indicator (starts the group)
                        ow = ipool.tile([16, 256], BF16,
                                        name=f"ow_{k}_{w}", tag="ow")
                        nc.sync.dma_start(
                            ow[:], shard[:, 256 * w : 256 * w + 256]
                        )
                        owv = ow[:].rearrange("p (e w) -> p e w", w=2)
                        tps = tppool.tile([128, 32], BF16, space="PSUM",
                                          name=f"tps_{k}_{w}", tag="tp")
                        for wsel in range(2):
                            nc.tensor.matmul(
                                tps[:, 16 * wsel : 16 * wsel + 16],
                                lhsT=owv[:, :, wsel],
                                rhs=id16_sb[:],
                                is_transpose=True, start=True,
                                stop=(wsel == 1),
                                skip_group_check=True,
                            )
                        rbs = rpool.tile([128, 32], BF16,
                                         name=f"rbs_{k}_{w}", tag="rb")
                        nc.vector.tensor_copy(rbs[:], tps[:])
                        nc.tensor.matmul(
                            ps[:], lhsT=id128_sb[:], rhs=rbs[:],
                            start=True, stop=False, skip_group_check=True,
                        )
                        n_ops_lw = 8 * len(ops_by_w[lw])
                        done = 0
                        for ck in ops_by_w[lw]:
                            for bp in range(4):
                                key = (bp, ck)
                                if key not in rhs_of_chunk:
                                    tp = tppool.tile(
                                        [128, 64], BF16, space="PSUM",
                                        name=f"tp_{k}_{t}_{bp}_{ck}",
                                        tag="tp",
                                    )
                                    for wsel in range(2):
                                        nc.tensor.matmul(
                                            tp[:, 32 * wsel : 32 * wsel + 32],
                                            lhsT=gsv[
                                                32 * bp : 32 * bp + 32,
                                                128 * ck : 128 * ck + 128,
                                                wsel,
                                            ],
                                            rhs=id32_sb[
                                                32 * bp : 32 * bp + 32, :
                                            ],
                                            is_transpose=True,
                                            start=True, stop=(wsel == 1),
                                            skip_group_check=True,
                                            tile_position=(32 * bp, 0),
                                        )
                                    # [wsel(2) x (half(2) x 16)] -> per-band
                                    tpv = tp[:].rearrange(
                                        "p (q h r) -> p h q r", q=2, h=2
                                    )
                                    rbs2 = []
                                    for h in range(2):
                                        rb = rpool.tile(
                                            [128, 32], BF16,
                                            name=f"rb_{k}_{t}_{bp}_{ck}_{h}",
                                            tag="rb",
                                        )
                                        nc.vector.tensor_copy(
                                            rb[:].rearrange(
                                                "p (q r) -> p q r", r=16
                                            ),
                                            tpv[:, h],
                                        )
                                        rbs2.append(rb)
                                    rhs_of_chunk[key] = rbs2
                                rbs2 = rhs_of_chunk[key]
                                for h in range(2):
                                    ind = npool.tile(
                                        [128, 128], BF16,
                                        name=f"in_{op_idx}", tag="in",
                                    )
                                    nc.vector.tensor_scalar(
                                        out=ind[:],
                                        in0=iota_sb[:],
                                        scalar1=dstloc_sb[
                                            :, op_idx : op_idx + 1
                                        ],
                                        scalar2=None,
                                        op0=mybir.AluOpType.is_equal,
                                    )
                                    done += 1
                                    nc.tensor.matmul(
                                        ps[:],
                                        lhsT=ind[:],
                                        rhs=rbs2[h][:],
                                        start=False,
                                        stop=done == n_ops_lw,
                                        skip_group_check=True,
                                    )
                                    op_idx += 1
                        # flush this window
                        t1 = fpool.tile([128, 32], F32, name=f"t1_{k}_{w}",
                                        tag="t1")
                        nc.vector.tensor_scalar_mul(
                            t1[:], ps[:], dinvw[:, w : w + 1]
                        )
                        t2 = fpool.tile([128, 32], F32, name=f"t2_{k}_{w}",
                                        tag="t2")
                        nc.vector.tensor_add(
                            t2[:], t1[:], biasb_sb[:, 32 * k : 32 * k + 32]
                        )
                        zb = fpool.tile([128, 32], BF16, name=f"zb_{k}_{w}",
                                        tag="zb")
                        nc.vector.tensor_scalar_max(zb[:], t2[:], 0.0)
                        zp = zppool.tile([32, 128], BF16, space="PSUM",
                                         name=f"zp_{k}_{w}", tag="zp")
                        nc.tensor.matmul(
                            zp[:], lhsT=zb[:], rhs=id128_sb[:],
                            is_transpose=True, start=True, stop=True,
                            skip_group_check=True,
                        )
                        nc.vector.tensor_copy(
                            xT[:, 128 * w : 128 * w + 128], zp[:]
                        )

            # ---- final linear + log_softmax (node-lane, strided groups) ----
            NG = WIN // 128
            xTg = xT[:].rearrange("f (p g) -> f p g", g=NG)
            for g in range(NG):
                ps = ppool.tile([128, 6], F32, space="PSUM",
                                name=f"psf_{g}", tag="ps")
                nc.tensor.matmul(
                    ps[:], lhsT=xTg[:, :, g : g + 1], rhs=Wf_sb[:],
                    start=True, stop=True,
                )
                logits = fpool.tile([128, 6], F32, name=f"lg_{g}", tag="lg")
                nc.vector.tensor_add(logits[:], ps[:], bfb_sb[:])
                m = fpool.tile([128, 1], F32, name=f"m_{g}", tag="m")
                nc.vector.tensor_reduce(
                    out=m[:], in_=logits[:],
                    axis=mybir.AxisListType.X, op=mybir.AluOpType.max,
                )
                negm = fpool.tile([128, 1], F32, name=f"nm_{g}", tag="nm")
                nc.vector.tensor_scalar_mul(negm[:], m[:], -1.0)
                e = fpool.tile([128, 6], F32, name=f"e_{g}", tag="e")
                s = fpool.tile([128, 1], F32, name=f"s_{g}", tag="s")
                nc.scalar.activation(
                    out=e[:], in_=logits[:],
                    func=mybir.ActivationFunctionType.Exp,
                    bias=negm[:], scale=1.0, accum_out=s[:],
                )
                ls = fpool.tile([128, 1], F32, name=f"ls_{g}", tag="ls")
                nc.scalar.activation(
                    out=ls[:], in_=s[:], func=mybir.ActivationFunctionType.Ln
                )
                shift = fpool.tile([128, 1], F32, name=f"sh_{g}", tag="sh")
                nc.vector.tensor_sub(shift[:], negm[:], ls[:])
                nc.vector.tensor_scalar_add(
                    out_sb[:, 6 * g : 6 * (g + 1)], logits[:], shift[:]
                )

            nc.sync.dma_start(
                out_dram.ap().rearrange("(p g) f -> p (g f)", g=NG),
                out_sb[:],
            )

    nc.compile()
    return nc


def kernel(x, edge_index, W1, b1, W2, b2, W3, b3, Wf, bf):
    x = np.asarray(x, dtype=np.float32)
    n_nodes = x.shape[0]
    pl = _build_plan(np.asarray(edge_index), n_nodes)
    nc = _build_kernel(pl)
    in_maps = _make_in_maps(pl, x, W1, b1, W2, b2, W3, b3, Wf, bf)

    res = run_bass_kernel_spmd(nc, in_maps, core_ids=list(range(pl.n_cores)))

    LAST_RUN_INFO.clear()
    LAST_RUN_INFO["exec_time_ns"] = res.exec_time_ns
    LAST_RUN_INFO["mean_exec_time_ns"] = res.mean_exec_time_ns

    outs = [res.results[c]["out"] for c in range(pl.n_cores)]
    return _assemble_output(pl, outs, d_out=6)
